# revision 23
# baseline (speedup 1.0000x reference)
"""Trainium2 Bass kernel for nn_Encoder_Block (B=2,S=2048,E=1024,H=16,D=64,FE=4).

Sharding: 8 NeuronCores, no collectives. Cores 0-3 take batch 0, cores 4-7
batch 1; each core owns a 512-query slice and runs the full encoder block
for those queries (it loads all keys/values of its batch plus all weights).

Attention, per head (all fp16 hi/lo split for fp32-grade scores at bf16 PE
rate; Wq,Wk folded into q' = q @ Wq.T@Wk on host; Wv folded into Wfc):
  max-pass: scores ~ q_hi.k_hi in [q,k] layout; per-chunk-pair running
            max via fused DVE tensor_tensor_reduce; -m reaches qaug row 64
            (fp16) through a DRAM transpose bounce.  m's exactness is
            irrelevant - it cancels in softmax; only overflow margin counts.
  exp-pass: per k-tile two accumulating matmuls
              [k_hi; ones] x [q_hi; -m]  +  [k_lo; k_hi] x [q_hi; q_lo]
            = k.q' - m  (only k_lo.q_lo dropped), then one ACT exp ->
            attnT fp16; ov accumulates vaug^T @ attnT ([65,q]; row 64 = Z).
  drain: zinv = 1/Z (DVE) -> broadcast via 1-contract matmul (PSUM) ->
         DVE multiply into the persistent fc lhsT tile (bf16).
Then fc (+queries+bfc residual via host-folded bias), LN1, PE transpose,
FFN1+relu, FFN2(+b2), residual, LN2.  Weights pre-tiled/cast on host.
"""
import os
import sys
import math
from contextlib import ExitStack

os.environ.setdefault("NEURON_RT_RESET_CORES", "1")
sys.path.insert(0, "/opt/trn_rl_repo")

import numpy as np
import concourse.bass as bass
import concourse.tile as tile
from concourse import mybir

F32 = mybir.dt.float32
F16 = mybir.dt.float16
BF16 = mybir.dt.bfloat16
AX = mybir.AxisListType.X
AC = mybir.AxisListType.C
AF = mybir.ActivationFunctionType
OP = mybir.AluOpType


class Cfg:
    def __init__(self, S=2048, E=1024, H=16, D=64, FE=4, T=512, eps=1e-5):
        self.S, self.E, self.H, self.D, self.FE, self.T, self.eps = S, E, H, D, FE, T, eps
        assert D == 64 and E == H * D
        self.KT = S // 128            # k partition-tiles
        self.QT = T // 128            # q tiles (per core)
        self.ET = E // 128            # e tiles
        self.ZT = FE * E // 128       # ffn hidden tiles
        self.CH = min(512, S)         # k chunk for DVE-head max pass
        self.NCH = S // self.CH
        self.EC = min(512, E)         # e moving chunk
        self.NEC = E // self.EC
        self.scale = math.sqrt(float(S))

    def perm(self):
        # pass-2 query order j <-> original query (j % QT)*128 + j // QT
        j = np.arange(self.T)
        return (j % self.QT) * 128 + j // self.QT


def _layernorm(nc, pool, x_ap, out_ap, g_b, b_b, eps_t, c, eng=None):
    """LayerNorm over the free dim (E) of x_ap [128, E] -> out_ap.

    `eng` picks the engine (nc.vector / nc.gpsimd) for the big elementwise
    tail ops so consecutive layernorms can alternate DVE and Pool."""
    E = c.E
    nsub = (E + 511) // 512
    stats = pool.tile([128, nsub, 6], F32, tag="ln_stats")
    xr = x_ap.rearrange("p (n s) -> p n s", n=nsub)
    for i in range(nsub):
        nc.vector.bn_stats(stats[:, i, :], xr[:, i, :])
    mv = pool.tile([128, 2], F32, tag="ln_mv")
    nc.vector.bn_aggr(mv[:], stats[:])
    rstd = pool.tile([128, 1], F32, tag="ln_rstd")
    nc.scalar.activation(rstd[:], mv[:, 1:2], AF.Sqrt, bias=eps_t[:], scale=1.0)
    nc.vector.reciprocal(rstd[:], rstd[:])
    t1 = pool.tile([128, E], F32, tag="ln_t1")
    t2 = pool.tile([128, E], F32, tag="ln_t2")
    # the per-partition-scalar op (TensorScalarPtr) only encodes on DVE;
    # plain tensor_tensor ops can ride Pool
    nc.vector.scalar_tensor_tensor(
        t1[:], x_ap, mv[:, 0:1], rstd[:].to_broadcast([128, E]),
        OP.subtract, OP.mult)
    if eng == "split":
        # halves run on DVE and Pool concurrently (tail-latency critical)
        half = E // 2
        for e, sl in ((nc.vector, slice(0, half)), (nc.gpsimd, slice(half, E))):
            e.tensor_tensor(t2[:, sl], t1[:, sl], g_b[:, sl], OP.mult)
            e.tensor_tensor(out_ap[:, sl], t2[:, sl], b_b[:, sl], OP.add)
        return
    if eng is None:
        eng = nc.vector
    eng.tensor_tensor(t2[:], t1[:], g_b[:], OP.mult)
    eng.tensor_tensor(out_ap, t2[:], b_b[:], OP.add)


def build_nc(c: Cfg):
    """Build the single-core program (pure SPMD — all cores run this)."""
    nc = bass.Bass()
    S, E, H, D, T = c.S, c.E, c.H, c.D, c.T

    dp = nc.declare_dram_parameter
    # per head: rows 0:64 = k_lo^T, rows 64:128 = k_hi^T (fp16 hi/lo split)
    khl_d = dp("khl", [H, 128, S], F16, isOutput=False)
    # per head: rows 0:64 = k_hi^T, row 64 = ones (host-baked aug row)
    ka_d = dp("ka", [H, 65, S], F16, isOutput=False)
    # per head, perm-order cols: rows 0:64 = q'_hi^T, 64:128 = q'_lo^T
    qhl_d = dp("qhl", [H, 128, T], F16, isOutput=False)
    qh_d = dp("qh", [E, T], F16, isOutput=False)             # q'_hi^T orig order
    qnat_d = dp("qnat", [T, E], F16, isOutput=False)         # queries+bfc (perm)
    v_d = dp("vv", [H, 128, c.KT, 65], F16, isOutput=False)  # v tiles + ones col
    wfc_d = dp("wfc", [128, c.ET, E], BF16, isOutput=False)  # Wfc_v^T tiled
    w1_d = dp("w1", [c.ZT, 128, E], BF16, isOutput=False)    # per zt: [e part, z]
    b1_d = dp("b1", [128, c.ZT], F32, isOutput=False)
    w2_d = dp("w2", [c.ZT, 128, E], BF16, isOutput=False)    # per zt: [z part, e]
    b2_d = dp("b2", [1, E], BF16, isOutput=False)
    g1_d = dp("g1", [1, E], F32, isOutput=False)
    be1_d = dp("be1", [1, E], F32, isOutput=False)
    g2_d = dp("g2", [1, E], F32, isOutput=False)
    be2_d = dp("be2", [1, E], F32, isOutput=False)
    out_d = dp("out", [T, E], F32, isOutput=True)            # perm rows

    with tile.TileContext(nc) as tc, ExitStack() as ctx:
        persist = ctx.enter_context(tc.tile_pool(name="persist", bufs=1))

        def bcast128_alloc(src_ap, nm, dtype=F32):
            t = persist.tile([128, src_ap.shape[1]], dtype, name=nm, tag=nm)
            src_b = bass.AP(tensor=src_ap.tensor, offset=src_ap.offset,
                            ap=[[0, 128]] + list(src_ap.ap[1:]))
            return t, src_b

        g1_b, g1_src = bcast128_alloc(g1_d[:], "g1b")
        be1_b, be1_src = bcast128_alloc(be1_d[:], "be1b")
        g2_b, g2_src = bcast128_alloc(g2_d[:], "g2b")
        be2_b, be2_src = bcast128_alloc(be2_d[:], "be2b")

        from concourse.masks import make_identity
        ident = persist.tile([128, 128], BF16)
        make_identity(nc, ident[:])

        eps_t = persist.tile([128, 1], F32)
        nc.vector.memset(eps_t[:], c.eps)

        ones_bf = persist.tile([1, 128], BF16)
        nc.vector.memset(ones_bf[:], 1.0)

        # (wfc/b1/b2 DMAs are issued after the first attention head's loads
        #  so they don't delay PE startup on the shared DMA path)
        wfc_t = persist.tile([128, c.ET, E], BF16)
        b1_t = persist.tile([128, c.ZT], F32)
        b2_t = persist.tile([1, E], BF16)

        ovT_pack = persist.tile([128, c.ET, T], BF16)
        h_sb = persist.tile([128, c.QT, E], F32)
        hT_bf = persist.tile([128, c.ET, T], BF16)
        x_sb = persist.tile([128, c.QT, E], F32)

        # =================== ATTENTION ===================
        with ExitStack() as actx:
            kB_p = actx.enter_context(tc.tile_pool(name="kB", bufs=3))
            kA_p = actx.enter_context(tc.tile_pool(name="kA", bufs=3))
            vv_p = actx.enter_context(tc.tile_pool(name="vv", bufs=3))
            rhs_p = actx.enter_context(tc.tile_pool(name="rhs", bufs=3))
            qaug_p = actx.enter_context(tc.tile_pool(name="qaug", bufs=3))
            qh_p = actx.enter_context(tc.tile_pool(name="qhp", bufs=3))
            attn_p = actx.enter_context(tc.tile_pool(name="attn", bufs=4))
            sm_p = actx.enter_context(tc.tile_pool(name="sm", bufs=2))
            zi_p = actx.enter_context(tc.tile_pool(name="zi", bufs=2))
            zdr_p = actx.enter_context(tc.tile_pool(name="zdr", bufs=2, space="DRAM"))
            mm_ps = actx.enter_context(tc.tile_pool(name="mm_ps", bufs=4, space="PSUM"))
            p2_ps = actx.enter_context(tc.tile_pool(name="p2_ps", bufs=2, space="PSUM"))
            ov_ps = actx.enter_context(tc.tile_pool(name="ov_ps", bufs=2, space="PSUM"))

            # staged state from the max-pass of head h, consumed by exp(h)
            stage = [None] * H

            def max_pass(h):
                """Load head h inputs; compute -m into qaug row 64."""
                # max-pass dependencies load first so PE can start early
                qh_t = qh_p.tile([64, T], F16, tag="qht")
                nc.sync.dma_start(qh_t[:], qh_d[h * D:(h + 1) * D, :])
                tileA = kA_p.tile([65, S], F16, tag="kA")
                nc.sync.dma_start(tileA[:], ka_d[h, :, :])

                # [q,k]-layout max pass: scores ~ q_hi.k_hi; per-chunk-pair
                # fused max via tensor_tensor_reduce with chained running max
                m16 = sm_p.tile([128, c.QT], F16, tag="m16")
                for qt in range(c.QT):
                    mtmp = sm_p.tile([128, max(c.NCH, 2)], F32, tag="mtmp")
                    for j in range(c.NCH):
                        jsl = slice(j * c.CH, (j + 1) * c.CH)
                        sps = mm_ps.tile([128, c.CH], F32, tag="mmps")
                        nc.tensor.matmul(
                            sps[:], qh_t[:, qt * 128:(qt + 1) * 128],
                            tileA[:64, jsl], start=True, stop=True)
                        nc.vector.reduce_max(mtmp[:, j:j + 1], sps[:], axis=AX)
                    run = sm_p.tile([128, 1], F32, tag="mrun")
                    nc.vector.reduce_max(run[:], mtmp[:, :c.NCH], axis=AX)
                    nc.gpsimd.tensor_scalar_mul(m16[:, qt:qt + 1], run[:], -1.0)

                # exp-pass inputs stream behind the max-pass compute
                tileB = kB_p.tile([128, S], F16, tag="kB")
                nc.sync.dma_start(tileB[:], khl_d[h, :, :])
                rhsB = rhs_p.tile([128, T], F16, tag="rhs")
                nc.sync.dma_start(rhsB[:], qhl_d[h, :, :])
                qaug = qaug_p.tile([65, T], F16, tag="qaug")
                nc.sync.dma_start(qaug[:64, :], qhl_d[h, 0:64, :])
                vaug = vv_p.tile([128, c.KT, 65], F16, tag="vaug")
                nc.sync.dma_start(vaug[:], v_d[h, :, :, :])
                # -m transpose bounce (emitted after this head's loads so the
                # in-order SP queue never head-blocks a load behind its wait)
                m_dram = zdr_p.tile([128, c.QT], F16, tag="mdram")
                nc.sync.dma_start(m_dram[:], m16[:])
                nc.sync.dma_start(qaug[64:65, :],
                                  m_dram[:].rearrange("r qt -> (r qt)")[None, :])
                return tileB, tileA, rhsB, qaug, vaug

            def exp_pass(h):
                tileB, tileA, rhsB, qaug, vaug = stage[h]
                ovp = ov_ps.tile([65, T], F32, tag="ovps")
                for t in range(c.KT):
                    tsl = slice(t * 128, (t + 1) * 128)
                    p2 = p2_ps.tile([128, T], F32, tag="p2ps")
                    nc.tensor.matmul(p2[:], tileA[:, tsl], qaug[:],
                                     start=True, stop=False)
                    nc.tensor.matmul(p2[:], tileB[:, tsl], rhsB[:],
                                     start=False, stop=True)
                    at = attn_p.tile([128, T], F16, tag="attnT")
                    nc.scalar.activation(at[:], p2[:], AF.Exp,
                                         bias=0.0, scale=c.scale)
                    nc.tensor.matmul(ovp[:], vaug[:, t, :], at[:],
                                     start=(t == 0), stop=(t == c.KT - 1),
                                     skip_group_check=True)
                # 1/Z -> DRAM -> partition-broadcast back (only one PSUM
                # operand is legal per ALU instruction, so the multiply reads
                # zinv from SBUF)
                zrow = zi_p.tile([1, T], F16, tag="zrow")
                with nc.allow_low_precision(reason="1/Z to fp16 (rel 2^-11)"):
                    nc.vector.reciprocal(zrow[:], ovp[64:65, :])
                zdr = zdr_p.tile([1, T], F16, tag="zdr")
                nc.sync.dma_start(zdr[:], zrow[:])
                zinv_b = zi_p.tile([64, T], F16, tag="zinv")
                zsrc = zdr[:]
                nc.sync.dma_start(
                    zinv_b[:],
                    bass.AP(tensor=zsrc.tensor, offset=zsrc.offset,
                            ap=[[0, 64]] + list(zsrc.ap[1:])))
                po = (h % 2) * 64
                nc.vector.tensor_tensor(
                    ovT_pack[po:po + 64, h // 2, :], ovp[:64, :], zinv_b[:], OP.mult)

            stage[0] = max_pass(0)
            stage[1] = max_pass(1)
            # fc/FFN/LN constants load behind the first two heads' input DMAs
            nc.sync.dma_start(wfc_t[:], wfc_d[:])
            nc.sync.dma_start(b1_t[:], b1_d[:])
            nc.sync.dma_start(b2_t[:], b2_d[:])
            nc.sync.dma_start(g1_b[:], g1_src)
            nc.sync.dma_start(be1_b[:], be1_src)
            nc.sync.dma_start(g2_b[:], g2_src)
            nc.sync.dma_start(be2_b[:], be2_src)
            for h in range(H):
                if h + 2 < H:
                    stage[h + 2] = max_pass(h + 2)
                exp_pass(h)
                stage[h] = None

        # =================== FC + LN1 + transpose(h) ===================
        with ExitStack() as fctx:
            qn_p = fctx.enter_context(tc.tile_pool(name="qn", bufs=2))
            st_p = fctx.enter_context(tc.tile_pool(name="st", bufs=2))
            fc_ps = fctx.enter_context(tc.tile_pool(name="fc_ps", bufs=2, space="PSUM"))
            tr_ps = fctx.enter_context(tc.tile_pool(name="tr_ps", bufs=2, space="PSUM"))

            for qt in range(c.QT):
                qsl = slice(qt * 128, (qt + 1) * 128)
                hpre = st_p.tile([128, E], F32, tag="hpre")
                qn = qn_p.tile([128, E], F16, tag="qn")
                nc.sync.dma_start(qn[:], qnat_d[qsl, :])
                for ec in range(c.NEC):
                    esl = slice(ec * c.EC, (ec + 1) * c.EC)
                    aps = fc_ps.tile([128, c.EC], F32, tag="fcps")
                    for dt in range(c.ET):
                        nc.tensor.matmul(aps[:], ovT_pack[:, dt, qsl],
                                         wfc_t[:, dt, esl],
                                         start=(dt == 0), stop=(dt == c.ET - 1))
                    nc.vector.scalar_tensor_tensor(
                        hpre[:, esl], aps[:], 1.0, qn[:, esl],
                        OP.bypass, OP.add)

                _layernorm(nc, st_p, hpre[:], h_sb[:, qt, :], g1_b, be1_b, eps_t, c,
                           eng=(nc.vector if qt % 2 == 0 else nc.gpsimd))
                hbf = st_p.tile([128, E], BF16, tag="hbf")
                nc.scalar.copy(hbf[:], h_sb[:, qt, :])
                for et in range(c.ET):
                    tps = tr_ps.tile([128, 128], BF16, tag="trps")
                    nc.tensor.transpose(tps[:], hbf[:, et * 128:(et + 1) * 128],
                                        ident[:])
                    nc.vector.tensor_copy(hT_bf[:, et, qsl], tps[:])

        # =================== FFN + LN2 ===================
        with ExitStack() as nctx:
            w_p = nctx.enter_context(tc.tile_pool(name="wstream", bufs=4))
            z_p = nctx.enter_context(tc.tile_pool(name="zrel", bufs=1))
            ln_p = nctx.enter_context(tc.tile_pool(name="lnp", bufs=2))
            z1_ps = nctx.enter_context(tc.tile_pool(name="z1_ps", bufs=2, space="PSUM"))
            x2_ps = nctx.enter_context(
                tc.tile_pool(name="x2_ps", bufs=c.QT, space="PSUM"))

            z1rel = z_p.tile([128, c.ZT, T], BF16, tag="z1rel")
            for zt in range(c.ZT):
                w1t = w_p.tile([128, E], BF16, tag="w1t")
                nc.sync.dma_start(w1t[:], w1_d[zt, :, :])
                zps = z1_ps.tile([128, T], F32, tag="z1ps")
                for et in range(c.ET):
                    nc.tensor.matmul(zps[:], w1t[:, et * 128:(et + 1) * 128],
                                     hT_bf[:, et, :],
                                     start=(et == 0), stop=(et == c.ET - 1))
                nc.scalar.activation(z1rel[:, zt, :], zps[:], AF.Relu,
                                     bias=b1_t[:, zt:zt + 1], scale=1.0)

            for ec in range(c.NEC):
                esl = slice(ec * c.EC, (ec + 1) * c.EC)
                xps = [x2_ps.tile([128, c.EC], F32, tag="x2ps", name=f"x2ps_{qt}")
                       for qt in range(c.QT)]
                for zt in range(c.ZT):
                    w2t = w_p.tile([128, c.EC], BF16, tag="w2t")
                    nc.sync.dma_start(w2t[:], w2_d[zt, :, esl])
                    for qt in range(c.QT):
                        nc.tensor.matmul(
                            xps[qt][:], z1rel[:, zt, qt * 128:(qt + 1) * 128],
                            w2t[:], start=(zt == 0), stop=False)
                for qt in range(c.QT):
                    nc.tensor.matmul(xps[qt][:], ones_bf[:, :128], b2_t[:, esl],
                                     start=False, stop=True)
                    nc.vector.scalar_tensor_tensor(
                        x_sb[:, qt, esl], xps[qt][:], 1.0, h_sb[:, qt, esl],
                        OP.bypass, OP.add)

            for qt in range(c.QT):
                outt = ln_p.tile([128, E], F32, tag="outt")
                _layernorm(nc, ln_p, x_sb[:, qt, :], outt[:], g2_b, be2_b, eps_t, c,
                           eng="split")
                nc.sync.dma_start(out_d[qt * 128:(qt + 1) * 128, :], outt[:])

    return nc


def _split_waits(nc, maxw=1):
    """walrus in this toolchain only accepts 1 sync-wait per instruction on
    several formats; move excess waits onto preceding same-engine NoOps."""
    ctr = 0
    for f in nc.m.functions:
        for bb in f.blocks:
            out = []
            for inst in bb.instructions:
                si = getattr(inst, "sync_info", None)
                if si is not None and si.on_wait and len(si.on_wait) > maxw:
                    waits = list(si.on_wait)
                    head, tail = waits[:-maxw], waits[-maxw:]
                    for i in range(0, len(head), maxw):
                        ctr += 1
                        out.append(mybir.InstNoOp(
                            name=f"waitsplit_{ctr}", engine=inst.engine,
                            ins=[], outs=[],
                            sync_info=mybir.SyncInfo(
                                on_wait=list(head[i:i + maxw]), on_update=[]),
                        ))
                    si.on_wait = tail
                out.append(inst)
            bb.instructions[:] = out


# ======================= host side =======================

def host_prep(c: Cfg, inputs, core):
    """Build the per-core input map (numpy only)."""
    B = inputs["queries"].shape[0]
    cores_per_batch = 8 // B if B <= 8 else 1
    b = core // cores_per_batch
    slot = core % cores_per_batch
    T = c.T
    perm = c.perm()

    q = np.asarray(inputs["queries"][b], np.float32)       # [S, E]
    k = np.asarray(inputs["keys"][b], np.float32)
    v = np.asarray(inputs["values"][b], np.float32)
    qs = q[slot * T:(slot + 1) * T]                        # [T, E]

    Wq = np.asarray(inputs["Wq"], np.float64)
    Wk = np.asarray(inputs["Wk"], np.float64)
    Wv = np.asarray(inputs["Wv"], np.float64)
    Wfc = np.asarray(inputs["Wfc"], np.float64)            # [E, E]
    W1 = np.asarray(inputs["W1"], np.float64)              # [FE*E, E]
    W2 = np.asarray(inputs["W2"], np.float64)              # [E, FE*E]

    # fold Wq/Wk into the queries: q'_h = q_h @ (Wq.T @ Wk); scores = q' @ k^T
    A_mid = Wq.T @ Wk
    E_, H_, D_ = c.E, c.H, c.D
    qp = np.empty((T, E_), np.float64)
    for h in range(H_):
        qp[:, h * D_:(h + 1) * D_] = qs[:, h * D_:(h + 1) * D_].astype(np.float64) @ A_mid
    qp = qp.astype(np.float32)
    qp_h = qp.astype(np.float16)
    qp_l = (qp - qp_h.astype(np.float32)).astype(np.float16)
    k_h = k.astype(np.float16)
    k_l = (k - k_h.astype(np.float32)).astype(np.float16)

    # khl[h]: rows 0:64 = k_lo^T, rows 64:128 = k_hi^T
    khl = np.empty((c.H, 128, c.S), np.float16)
    for h in range(H_):
        khl[h, :64] = k_l[:, h * D_:(h + 1) * D_].T
        khl[h, 64:] = k_h[:, h * D_:(h + 1) * D_].T
    # ka[h]: rows 0:64 = k_hi^T, row 64 = ones (aug row for the -m matmul)
    ka = np.ones((c.H, 65, c.S), np.float16)
    ka[:, :64] = khl[:, 64:]

    # qhl[h] (perm order): rows 0:64 = q'_hi^T, rows 64:128 = q'_lo^T
    qhTp = qp_h[perm].T                                    # [E, T] perm order
    qlTp = qp_l[perm].T
    qhl = np.empty((c.H, 128, T), np.float16)
    for h in range(H_):
        qhl[h, :64] = qhTp[h * D_:(h + 1) * D_]
        qhl[h, 64:] = qlTp[h * D_:(h + 1) * D_]

    qhT = np.ascontiguousarray(qp_h.T)                     # [E, T] orig order

    # Wfc_v[e, h*64+d] = sum_dd Wfc[e, h*64+dd] * Wv[dd, d]
    E, H, D = c.E, c.H, c.D
    wfcv = np.empty((E, E), np.float64)
    for h in range(H):
        wfcv[:, h * D:(h + 1) * D] = Wfc[:, h * D:(h + 1) * D] @ Wv
    # rhs tiles: wfc_prep[p, dt, e] = Wfc_v[e, dt*128+p]
    wfc_prep = np.ascontiguousarray(
        wfcv.T.reshape(c.ET, 128, E).transpose(1, 0, 2)).astype(ml_bf16())

    # w1_prep[zt, p, et*128 + z] = W1[zt*128+z, et*128+p]
    w1r = W1.reshape(c.ZT, 128, c.ET, 128)                 # [zt, z, et, p]
    w1_prep = np.ascontiguousarray(
        w1r.transpose(0, 3, 2, 1).reshape(c.ZT, 128, E)).astype(ml_bf16())

    # w2_prep[zt, p, e] = W2[e, zt*128+p]
    w2r = W2.T.reshape(c.ZT, 128, E)                       # [zt, p, e]
    w2_prep = np.ascontiguousarray(w2r).astype(ml_bf16())

    b1 = np.asarray(inputs["b1"], np.float32)
    b1_prep = np.ascontiguousarray(b1.reshape(c.ZT, 128).T)  # [128, ZT]

    qnat = qs[perm].astype(np.float64) + np.asarray(inputs["bfc"], np.float64)[None, :]

    # v tiles with baked ones column: vtile[h, p, t, 0:64] = v[t*128+p, h*64+d]
    vtile = np.ones((c.H, 128, c.KT, 65), np.float16)
    v16 = v.astype(np.float16).reshape(c.KT, 128, c.H, c.D)   # [t, p, h, d]
    vtile[:, :, :, :64] = v16.transpose(2, 1, 0, 3)

    return {
        "khl": khl,
        "ka": ka,
        "qhl": qhl,
        "qh": qhT,
        "qnat": np.ascontiguousarray(qnat.astype(np.float16)),
        "vv": vtile,
        "wfc": wfc_prep,
        "w1": w1_prep,
        "b1": b1_prep,
        "w2": w2_prep,
        "b2": np.asarray(inputs["b2"], np.float32)[None, :].astype(ml_bf16()),
        "g1": np.asarray(inputs["ln1_g"], np.float32)[None, :],
        "be1": np.asarray(inputs["ln1_b"], np.float32)[None, :],
        "g2": np.asarray(inputs["ln2_g"], np.float32)[None, :],
        "be2": np.asarray(inputs["ln2_b"], np.float32)[None, :],
    }


def ml_bf16():
    import ml_dtypes
    return ml_dtypes.bfloat16


_CACHE = {}


def kernel(**inputs):
    """Full-input entry point: shard across 8 cores, run, gather."""
    c = Cfg()
    B, S, E = inputs["queries"].shape
    assert (B, S, E) == (2, c.S, c.E), (B, S, E)

    if "nc" not in _CACHE:
        nc = build_nc(c)
        _split_waits(nc)   # walrus wait-slot workaround (compile path only)
        _CACHE["nc"] = nc
    nc = _CACHE["nc"]

    in_maps = [host_prep(c, inputs, core) for core in range(8)]

    from concourse.bass_utils import run_bass_kernel_spmd
    res = run_bass_kernel_spmd(nc, in_maps, core_ids=list(range(8)))

    perm = c.perm()
    out = np.empty((B, S, E), np.float32)
    cores_per_batch = 4
    for core in range(8):
        b = core // cores_per_batch
        slot = core % cores_per_batch
        block = np.empty((c.T, E), np.float32)
        block[perm] = res.results[core]["out"]
        out[b, slot * c.T:(slot + 1) * c.T] = block
    return out
